# revision 6
# baseline (speedup 1.0000x reference)
"""Trainium2 Bass kernel for nn_CSFlow (RAFT-style correlation pyramid lookup).

Math restructure (exact up to fp16 rounding of stored corr values):
  - corr(q, pos) = <fmap1[:, q], fmap2[:, pos]> / sqrt(D). Pooling the corr
    volume over (i, j) == pooling fmap2 (linearity), so each pyramid level is
    its own matmul against a pooled fmap2.
  - All 81 lookup offsets of one query share the same fractional bilinear
    weights (integer offsets), so the lookup = gather of a 10x10 integer
    window + separable 2-tap blends with per-query weights.
  - The 10x10 window at a per-query position is fetched from an HBM scratch
    copy of that query's corr map with ONE indirect-DMA descriptor per
    (query, level): a contiguous band of 9*S+10 elements (S = inner-axis
    size) starting at the window origin. The data between window rows inside
    the band is simply never read (strided view). Out-of-range taps are
    zeroed exactly via host-precomputed masks folded into the stage-1 blend
    weights; band reads that spill outside a query's map hit neighbouring
    maps / pre-zeroed guard rows, so they are finite and masked.
  - Levels 0-2 store maps x-major (inner = y, size H_l >= 12); level 3
    stores y-major (inner = x, size 20) because H_3 = 6 < 10 would make the
    window view overlap. The host permutes level-3 output channels back.

Sharding: 8 cores x 1920 queries (B*H*W = 15360 split contiguously; cores
0-3 handle batch 0, cores 4-7 batch 1). kernel() takes full inputs and
returns the full output; everything device-side runs SPMD on 8 cores.
"""

import numpy as np

import concourse.bass as bass
import concourse.mybir as mybir
import concourse.tile as tile
from concourse import bacc
from concourse.bass_utils import run_bass_kernel_spmd
from concourse.masks import make_identity

# problem shape (hardcoded per harness contract)
B, D, H, W = 2, 256, 48, 160
NCORES = 8
QPC = (B * H * W) // NCORES      # 1920 queries per core
P = 128                          # queries per tile (partitions)
NT = QPC // P                    # 15 tiles per core
NLVL = 4
LH = [48, 24, 12, 6]
LW = [160, 80, 40, 20]
LHW = [LH[i] * LW[i] for i in range(NLVL)]           # 7680 1920 480 120
LOFF = [0, 7680, 9600, 10080]                        # col offset in f2 concat
NPOS = 10200
XMAJ = [True, True, True, False]                     # storage orientation
ST = [48, 24, 12, 20]                                # inner-axis size
BAND = [9 * s + 10 for s in ST]                      # 442 226 118 190
BANDT = [10 * s for s in ST]                         # band tile alloc
GUARD = [10 * s + 16 for s in ST]
SCRN = [GUARD[i] + QPC * LHW[i] + GUARD[i] + BAND[i] + 64 for i in range(NLVL)]

F16 = mybir.dt.float16
F32 = mybir.dt.float32
I32 = mybir.dt.int32

PSUM_CHUNK = 1024
MM_CHUNK = 512


def _chunks(total, step):
    return [(o, min(step, total - o)) for o in range(0, total, step)]


def build_nc():
    nc = bacc.Bacc("TRN2", target_bir_lowering=False, debug=False)

    f1t = nc.dram_tensor("f1t", [2, P, QPC], F32, kind="ExternalInput")
    f2t = nc.dram_tensor("f2t", [2, P, NPOS], F32, kind="ExternalInput")
    idxt = nc.dram_tensor("idxt", [P, NLVL * NT], I32, kind="ExternalInput")
    # stage-2 blend per-query scalars, 2 per (lvl,tile)
    wgtt = nc.dram_tensor("wgtt", [P, NLVL * NT * 2], F32, kind="ExternalInput")
    # stage-1 blend weights with validity masks folded in, 90 per (lvl,tile)
    my0t = nc.dram_tensor("my0t", [P, NLVL * NT * 90], F32, kind="ExternalInput")
    my1t = nc.dram_tensor("my1t", [P, NLVL * NT * 90], F32, kind="ExternalInput")
    outp = nc.dram_tensor("outp", [NLVL, 81, QPC], F32, kind="ExternalOutput")

    with tile.TileContext(nc) as tc:
        with (
            tc.tile_pool(name="dram", bufs=1, space="DRAM") as dpool,
            tc.tile_pool(name="const", bufs=1) as cpool,
            tc.tile_pool(name="corrchunk", bufs=4) as ckpool,
            tc.tile_pool(name="bands", bufs=3) as bpool,
            tc.tile_pool(name="blend", bufs=3) as blpool,
            tc.tile_pool(name="psum", bufs=3, space="PSUM") as pspool,
            tc.tile_pool(name="psumt", bufs=2, space="PSUM") as ptpool,
        ):
            # ---- constants / persistent tiles ----
            identity = cpool.tile([P, P], F32)
            make_identity(nc, identity)

            scr = [
                dpool.tile([SCRN[l]], F16, name=f"scr{l}") for l in range(NLVL)
            ]

            zguard = cpool.tile([1, 2048], F16)
            nc.vector.memset(zguard[:], 0.0)
            # zero head guard, tail guard(+band), and each tile-boundary
            # spill region so indirect gathers never read uninitialized HBM.
            for l in range(NLVL):
                g, hw = GUARD[l], LHW[l]
                tail = GUARD[l] + BAND[l] + 64
                spill = BAND[l] + 16
                nc.sync.dma_start(scr[l][0:g].unsqueeze(0), zguard[0:1, 0:g])
                nc.sync.dma_start(
                    scr[l][g + QPC * hw : g + QPC * hw + tail].unsqueeze(0),
                    zguard[0:1, 0:tail],
                )
                for t in range(1, NT):
                    nc.sync.dma_start(
                        scr[l][g + t * P * hw : g + t * P * hw + spill].unsqueeze(0),
                        zguard[0:1, 0:spill],
                    )

            f1sb = cpool.tile([P, 2 * QPC], F32)
            nc.sync.dma_start(f1sb[:, 0:QPC], f1t[0])
            nc.sync.dma_start(f1sb[:, QPC : 2 * QPC], f1t[1])
            f2sb0 = cpool.tile([P, NPOS], F32)
            f2sb1 = cpool.tile([P, NPOS], F32)
            nc.sync.dma_start(f2sb0[:], f2t[0])
            nc.sync.dma_start(f2sb1[:], f2t[1])
            idx_sb = cpool.tile([P, NLVL * NT], I32)
            nc.sync.dma_start(idx_sb[:], idxt[:])
            wgt_sb = cpool.tile([P, NLVL * NT * 2], F32)
            nc.sync.dma_start(wgt_sb[:], wgtt[:])
            my0_sb = cpool.tile([P, NLVL * NT * 90], F32)
            nc.sync.dma_start(my0_sb[:], my0t[:])
            my1_sb = cpool.tile([P, NLVL * NT * 90], F32)
            nc.sync.dma_start(my1_sb[:], my1t[:])

            ofin = [cpool.tile([81, QPC], F32, name=f"ofin{l}") for l in range(NLVL)]

            # ---- main loop over query tiles ----
            copy_rr = 0  # distribute ACT/DVE psum->sbuf copies
            for t in range(NT):
                # === corr matmuls -> psum -> sbuf fp16 -> HBM scratch ===
                for l in range(NLVL):
                    hw = LHW[l]
                    for coff, csz in _chunks(hw, PSUM_CHUNK):
                        ps = pspool.tile([P, PSUM_CHUNK], F32, name="cps")[:, :csz]
                        for k in range(2):
                            f2sb = f2sb0 if k == 0 else f2sb1
                            for soff, ssz in _chunks(csz, MM_CHUNK):
                                nc.tensor.matmul(
                                    ps[:, soff : soff + ssz],
                                    f1sb[:, k * QPC + t * P : k * QPC + (t + 1) * P],
                                    f2sb[
                                        :,
                                        LOFF[l] + coff + soff : LOFF[l]
                                        + coff
                                        + soff
                                        + ssz,
                                    ],
                                    start=(k == 0),
                                    stop=(k == 1),
                                )
                        ck = ckpool.tile([P, PSUM_CHUNK], F16, name="ck")[:, :csz]
                        if copy_rr % 3 == 0:
                            nc.vector.tensor_copy(ck, ps)
                        else:
                            nc.scalar.copy(ck, ps)
                        copy_rr += 1
                        base = GUARD[l] + t * P * hw
                        nc.sync.dma_start(
                            scr[l][base : base + P * hw]
                            .rearrange("(p x) -> p x", x=hw)[:, coff : coff + csz],
                            ck,
                        )

                # === gather bands + blend ===
                for l in range(NLVL):
                    s = ST[l]
                    band = bpool.tile(
                        [P, BANDT[l]], F16, name=f"band{l}", tag=f"band{l}"
                    )
                    nc.gpsimd.indirect_dma_start(
                        out=band[:, 0 : BAND[l]],
                        out_offset=None,
                        in_=scr[l][:].unsqueeze(1),
                        in_offset=bass.IndirectOffsetOnAxis(
                            ap=idx_sb[:, l * NT + t : l * NT + t + 1], axis=0
                        ),
                        element_offset=0,
                    )
                    # window views: g0[p,r,j] = band[p, r*s + j], g1 = j+1
                    bw = band[:].rearrange("p (r s) -> p r s", s=s)
                    g0 = bw[:, 0:10, 0:9]
                    g1 = bw[:, 0:10, 1:10]
                    c90 = (l * NT + t) * 90
                    m0 = my0_sb[:, c90 : c90 + 90].rearrange("p (r j) -> p r j", j=9)
                    m1 = my1_sb[:, c90 : c90 + 90].rearrange("p (r j) -> p r j", j=9)
                    t1 = blpool.tile([P, 90], F32, name="t1")
                    t1v = t1[:].rearrange("p (r j) -> p r j", j=9)
                    t2 = blpool.tile([P, 90], F32, name="t2")
                    t2v = t2[:].rearrange("p (r j) -> p r j", j=9)
                    # stage-1 blend along inner axis, masks folded into weights
                    nc.vector.tensor_tensor(
                        out=t1v, in0=g0, in1=m0, op=mybir.AluOpType.mult
                    )
                    nc.vector.tensor_tensor(
                        out=t2v, in0=g1, in1=m1, op=mybir.AluOpType.mult
                    )
                    nc.vector.tensor_add(out=t1[:], in0=t1[:], in1=t2[:])
                    # stage-2 blend along outer axis, per-query scalar weights
                    t1r = t1[:].rearrange("p (r j) -> p r j", j=9)
                    c2 = (l * NT + t) * 2
                    o = blpool.tile([P, 81], F32, name="o")
                    ov = o[:].rearrange("p (a j) -> p a j", j=9)
                    o2 = blpool.tile([P, 81], F32, name="o2")
                    o2v = o2[:].rearrange("p (a j) -> p a j", j=9)
                    nc.scalar.mul(ov, t1r[:, 0:9, :], wgt_sb[:, c2 : c2 + 1])
                    nc.vector.tensor_scalar(
                        o2v,
                        t1r[:, 1:10, :],
                        wgt_sb[:, c2 + 1 : c2 + 2],
                        None,
                        op0=mybir.AluOpType.mult,
                    )
                    nc.vector.tensor_add(out=o[:], in0=o[:], in1=o2[:])
                    # transpose [128q, 81] -> [81, 128q] and stash
                    pt = ptpool.tile([81, P], F32, name="pt")
                    nc.tensor.transpose(pt[:], o[:], identity[:])
                    nc.scalar.copy(ofin[l][:, t * P : (t + 1) * P], pt[:])

            for l in range(NLVL):
                nc.sync.dma_start(outp[l], ofin[l][:])

    nc.compile()
    return nc


# ---------------- host side ----------------

def _pool2(x):
    n, c, h, w = x.shape
    return x.reshape(n, c, h // 2, 2, w // 2, 2).mean(axis=(3, 5))


def _host_prep(fmap1, fmap2, coords):
    fmap1 = np.asarray(fmap1, np.float32)
    fmap2 = np.asarray(fmap2, np.float32)
    coords = np.asarray(coords, np.float32)
    scale = np.float32(1.0 / np.sqrt(D))

    # pooled fmap2 levels, flattened in storage orientation, scaled
    levels = []
    cur = fmap2 * scale
    for l in range(NLVL):
        if XMAJ[l]:
            levels.append(
                np.ascontiguousarray(cur.transpose(0, 1, 3, 2)).reshape(B, D, LHW[l])
            )
        else:
            levels.append(cur.reshape(B, D, LHW[l]))
        if l < NLVL - 1:
            cur = _pool2(cur)
    f2cat = np.concatenate(levels, axis=2)  # [B, D, NPOS]

    cx = coords[:, 0].reshape(-1)  # [B*H*W], query q = b*H*W + h*W + w
    cy = coords[:, 1].reshape(-1)
    nq = cx.shape[0]

    idx_all = np.zeros((NLVL, nq), np.int32)
    wgt_all = np.zeros((NLVL, nq, 2), np.float32)
    my0_all = np.zeros((NLVL, nq, 10, 9), np.float32)
    my1_all = np.zeros((NLVL, nq, 10, 9), np.float32)
    q_local = (np.arange(nq) % QPC).astype(np.int64)
    rr = np.arange(10)
    for l in range(NLVL):
        inv = np.float32(1.0 / (1 << l))
        x = cx * inv
        y = cy * inv
        x0 = np.floor(x)
        y0 = np.floor(y)
        wx = (x - x0).astype(np.float32)
        wy = (y - y0).astype(np.float32)
        x0c = np.clip(x0, -5, LW[l] + 4).astype(np.int64)
        y0c = np.clip(y0, -5, LH[l] + 4).astype(np.int64)
        vx = ((x0[:, None] + rr[None, :] - 4) >= 0) & (
            (x0[:, None] + rr[None, :] - 4) <= LW[l] - 1
        )  # [nq, 10] validity of x-tap x0-4+i
        vy = ((y0[:, None] + rr[None, :] - 4) >= 0) & (
            (y0[:, None] + rr[None, :] - 4) <= LH[l] - 1
        )
        if XMAJ[l]:
            # outer = x (weight wx), inner = y (weight wy)
            idx_all[l] = (
                GUARD[l] + q_local * LHW[l] + (x0c - 4) * LH[l] + (y0c - 4)
            ).astype(np.int32)
            wgt_all[l, :, 0] = 1.0 - wx
            wgt_all[l, :, 1] = wx
            m0 = vx[:, :, None] & vy[:, None, 0:9]
            m1 = vx[:, :, None] & vy[:, None, 1:10]
            my0_all[l] = m0 * (1.0 - wy)[:, None, None]
            my1_all[l] = m1 * wy[:, None, None]
        else:
            # outer = y (weight wy), inner = x (weight wx)
            idx_all[l] = (
                GUARD[l] + q_local * LHW[l] + (y0c - 4) * LW[l] + (x0c - 4)
            ).astype(np.int32)
            wgt_all[l, :, 0] = 1.0 - wy
            wgt_all[l, :, 1] = wy
            m0 = vy[:, :, None] & vx[:, None, 0:9]
            m1 = vy[:, :, None] & vx[:, None, 1:10]
            my0_all[l] = m0 * (1.0 - wx)[:, None, None]
            my1_all[l] = m1 * wx[:, None, None]

    def core_map(c):
        b = c // (NCORES // B)
        cl = c % (NCORES // B)
        sl = slice(c * QPC, (c + 1) * QPC)
        f1c = fmap1.reshape(B, D, H * W)[b][:, cl * QPC : (cl + 1) * QPC]
        return {
            "f1t": np.ascontiguousarray(f1c.reshape(2, P, QPC)),
            "f2t": np.ascontiguousarray(f2cat[b].reshape(2, P, NPOS)),
            "idxt": np.ascontiguousarray(
                idx_all[:, sl].reshape(NLVL, NT, P).transpose(2, 0, 1).reshape(P, -1)
            ),
            "wgtt": np.ascontiguousarray(
                wgt_all[:, sl].reshape(NLVL, NT, P, 2)
                .transpose(2, 0, 1, 3)
                .reshape(P, -1)
            ),
            "my0t": np.ascontiguousarray(
                my0_all[:, sl].reshape(NLVL, NT, P, 90)
                .transpose(2, 0, 1, 3)
                .reshape(P, -1)
            ),
            "my1t": np.ascontiguousarray(
                my1_all[:, sl].reshape(NLVL, NT, P, 90)
                .transpose(2, 0, 1, 3)
                .reshape(P, -1)
            ),
        }

    return [core_map(c) for c in range(NCORES)]


def assemble(results):
    out = np.empty((B, NLVL * 81, H * W), np.float32)
    for c in range(NCORES):
        b = c // (NCORES // B)
        lo = (c % (NCORES // B)) * QPC
        r = np.asarray(results[c]["outp"], np.float32).reshape(NLVL, 81, QPC)
        for l in range(NLVL):
            blk = r[l]
            if not XMAJ[l]:
                # stored channel order is bi*9+a; reference wants 9a+bi
                blk = blk.reshape(9, 9, QPC).transpose(1, 0, 2).reshape(81, QPC)
            out[b, l * 81 : (l + 1) * 81, lo : lo + QPC] = blk
    return out.reshape(B, NLVL * 81, H, W)


_NC_CACHE = {}


def get_nc():
    if "nc" not in _NC_CACHE:
        _NC_CACHE["nc"] = build_nc()
    return _NC_CACHE["nc"]


def kernel(fmap1, fmap2, coords):
    in_maps = _host_prep(fmap1, fmap2, coords)
    nc = get_nc()
    res = run_bass_kernel_spmd(nc, in_maps, core_ids=list(range(NCORES)))
    return assemble(res.results)
